# revision 18
# baseline (speedup 1.0000x reference)
"""GQA attention kernel for Trainium2, 8-core tensor-parallel.

Sharding: 8 cores = 2 batches x 4 KV-groups. Each core handles one
(batch, kv_group): projections for its 4 Q-heads + 1 KV-head, RoPE,
causal attention, and its row-shard of Wo -> partial [T, D] output.
Host sums the 4 partials per batch (the Wo all-reduce) at unshard.

V2: fp16 matmul operands throughout (same PE rate as f32r, half the
DMA/SBUF, 4x DVE modes). Attention in transposed orientation (S^T
tiles [s,q] from single K=128 matmuls). Softmax row-sums accumulate on
the vector engine into an SBUF f32 tile, reduced by ONE all-ones
stationary matmul per (j,head) which also broadcasts sums across
partitions. Diagonal blocks are trimmed to the causal region at
128-col granularity; the per-element causal mask is a single [128,128]
triangle multiply per diagonal block. exp() runs once per PAIR of
s-chunks over a 2-bank PSUM tile. PSUM->SBUF output copies run on the
Pool engine; output partials are written fp16 (host sums in f32).
"""
from contextlib import ExitStack

import numpy as np

import concourse.bass as bass
import concourse.mybir as mybir
import concourse.tile as tile
from concourse import bacc
from concourse.bass_utils import run_bass_kernel_spmd

B, T, D = 2, 2048, 2048
H, KV, HD = 16, 4, 128
R = H // KV                  # 4 query heads per kv head (per core)
GC = R * HD                  # 512 query-proj cols per core
THETA = 10000.0
TQ = 512                     # q-tile size
NJ = T // TQ                 # 4 q-tiles
ND = D // 128                # 16 contraction chunks
SCALE = float(HD) ** -0.5

F32 = mybir.dt.float32
F32R = mybir.dt.float32r
FP16 = mybir.dt.float16
AF = mybir.ActivationFunctionType

_CACHED_NC = None


def _build_nc():
    nc = bacc.Bacc("TRN2", target_bir_lowering=False, debug=False, num_devices=8)

    xT = nc.dram_tensor("xT", [ND, 128, T], FP16, kind="ExternalInput").ap()
    wq = nc.dram_tensor("wq", [128, ND * GC], FP16, kind="ExternalInput").ap()
    wk = nc.dram_tensor("wk", [128, ND * HD], FP16, kind="ExternalInput").ap()
    wv = nc.dram_tensor("wv", [128, ND * HD], FP16, kind="ExternalInput").ap()
    wo = nc.dram_tensor("wo", [128, R * D], FP16, kind="ExternalInput").ap()
    cosT = nc.dram_tensor("cosT", [HD, T], FP16, kind="ExternalInput").ap()
    sinT = nc.dram_tensor("sinT", [HD, T], FP16, kind="ExternalInput").ap()
    trimask = nc.dram_tensor("trimask", [128, 128], FP16, kind="ExternalInput").ap()
    out = nc.dram_tensor("out", [T, D], FP16, kind="ExternalOutput").ap()

    with tile.TileContext(nc) as tc, ExitStack() as ctx:
        res = ctx.enter_context(tc.tile_pool(name="res", bufs=1))
        sb = ctx.enter_context(tc.tile_pool(name="sb", bufs=2))
        pp = ctx.enter_context(tc.tile_pool(name="pp", bufs=2, space="PSUM"))

        # ---- resident weights / tables ----
        # j=0 activations interleave with the weight DMAs in consumption
        # order so the first matmuls start as early as possible.
        # j=0 activations: first 4 d-chunks as individual 2D DMAs (fine-grained
        # deps so the first A1 matmuls start ASAP), the rest as 3D gathers.
        wk_sb = res.tile([128, ND * HD], FP16)
        nc.sync.dma_start(wk_sb[:], wk[:])
        xt0_q = [sb.tile([128, 4, TQ], FP16, tag="xtq", bufs=4, name=f"xt0_q{qtr}")
                 for qtr in range(4)]
        for d in range(4):
            nc.sync.dma_start(xt0_q[0][:, d, :], xT[d, :, 0:TQ])
        cosj0 = sb.tile([128, TQ], FP16, tag="cos", bufs=2, name="cosj0")
        nc.sync.dma_start(cosj0[:], cosT[:, 0:TQ])
        sinj0 = sb.tile([128, TQ], FP16, tag="sin", bufs=2, name="sinj0")
        nc.sync.dma_start(sinj0[:], sinT[:, 0:TQ])
        for qtr in range(1, 4):
            nc.sync.dma_start(
                xt0_q[qtr][:], xT[qtr * 4:(qtr + 1) * 4, :, 0:TQ].transpose([1, 0, 2]))
        wv_sb = res.tile([128, ND * HD], FP16)
        nc.sync.dma_start(wv_sb[:], wv[:])
        wq_sb = res.tile([128, ND * GC], FP16)    # chunk d at cols [d*GC, (d+1)*GC)
        for part in range(4):
            nc.sync.dma_start(wq_sb[:, part * 4 * GC:(part + 1) * 4 * GC],
                              wq[:, part * 4 * GC:(part + 1) * 4 * GC])
        mask_sb = res.tile([128, 128], FP16)
        nc.sync.dma_start(mask_sb[:], trimask[:])
        wo_sb = res.tile([128, R * D], FP16)      # head h rows at cols [h*D, (h+1)*D)
        nc.sync.dma_start(wo_sb[:], wo[:])
        kT_sb = res.tile([128, T], FP16)          # K^T resident, filled per j
        v_sb = res.tile([128, T], FP16)           # V natural, chunk c at cols c*128
        ident = res.tile([128, 128], FP16)
        from concourse.masks import make_identity
        make_identity(nc, ident[:])
        ones_c = res.tile([128, 128], FP16)       # sigma-reduce+broadcast stationary
        nc.vector.memset(ones_c[:], 1.0)

        def c_block(o_t, q0p, qs, split_dma=False):
            # output projection for rows [q0p+qs*128, q0p+(qs+1)*128)
            ob = sb.tile([128, D], FP16, tag="ob", bufs=3)
            for n in range(NJ):
                pc = pp.tile([128, 512], F32, tag="pa", bufs=2)
                for h2 in range(R):
                    nc.tensor.matmul(
                        pc[:], o_t[h2][:, qs * 128:(qs + 1) * 128],
                        wo_sb[:, h2 * D + n * 512:h2 * D + (n + 1) * 512],
                        start=(h2 == 0), stop=(h2 == R - 1))
                if n % 2 == 0:
                    nc.scalar.copy(ob[:, n * 512:(n + 1) * 512], pc[:])
                else:
                    nc.vector.tensor_copy(ob[:, n * 512:(n + 1) * 512], pc[:])
                if split_dma:
                    nc.gpsimd.dma_start(
                        out[q0p + qs * 128:q0p + (qs + 1) * 128,
                            n * 512:(n + 1) * 512],
                        ob[:, n * 512:(n + 1) * 512])
            if not split_dma:
                nc.gpsimd.dma_start(
                    out[q0p + qs * 128:q0p + (qs + 1) * 128, :], ob[:])

        def rope(dst, ps, cosj, sinj):
            # dst = ps * cos + rotate_half(ps) * sin   (partition dim = head dim)
            # one scalar op moves PSUM->SBUF fp16; the rest is 4x-mode DVE.
            ps_sb = sb.tile([128, TQ], FP16, tag="ps_sb", bufs=2)
            nc.scalar.copy(ps_sb[:], ps[:])
            rot = sb.tile([128, TQ], FP16, tag="rot", bufs=2)
            nc.vector.tensor_scalar_mul(rot[0:64, :], ps_sb[64:128, :], -1.0)
            nc.vector.tensor_copy(rot[64:128, :], ps_sb[0:64, :])
            tmp = sb.tile([128, TQ], FP16, tag="ropetmp", bufs=2)
            nc.vector.tensor_mul(tmp[:], rot[:], sinj[:])
            nc.vector.tensor_mul(dst, ps_sb[:], cosj[:])
            nc.vector.tensor_add(dst, dst, tmp[:])

        prev_o, prev_q0 = None, 0
        for j in range(NJ):
            q0 = j * TQ
            # ---- stage inputs for this q/s tile ----
            if j == 0:
                xts = [xt0_q[d // 4][:, d % 4, :] for d in range(ND)]
                cosj, sinj = cosj0, sinj0
            else:
                xt_all = sb.tile([128, ND, TQ], FP16, tag="xt", bufs=2)
                nc.sync.dma_start(
                    xt_all[:], xT[:, :, q0:q0 + TQ].transpose([1, 0, 2]))
                xts = [xt_all[:, d, :] for d in range(ND)]
                cosj = sb.tile([128, TQ], FP16, tag="cos", bufs=2)
                nc.sync.dma_start(cosj[:], cosT[:, q0:q0 + TQ])
                sinj = sb.tile([128, TQ], FP16, tag="sin", bufs=2)
                nc.sync.dma_start(sinj[:], sinT[:, q0:q0 + TQ])

            # ---- A1: K^T and V^T for s-tile j ----
            k_ps = pp.tile([128, TQ], F32, tag="pa", bufs=2)
            vt_ps = pp.tile([128, TQ], F32, tag="pa", bufs=2)
            for d in range(ND):
                nc.tensor.matmul(k_ps[:], wk_sb[:, d * HD:(d + 1) * HD], xts[d],
                                 start=(d == 0), stop=(d == ND - 1))
            for d in range(ND):
                nc.tensor.matmul(vt_ps[:], wv_sb[:, d * HD:(d + 1) * HD], xts[d],
                                 start=(d == 0), stop=(d == ND - 1))
            rope(kT_sb[:, q0:q0 + TQ], k_ps, cosj, sinj)
            vt_sbt = sb.tile([128, TQ], FP16, tag="vtsb", bufs=2)
            nc.scalar.copy(vt_sbt[:], vt_ps[:])
            for c4 in range(4):
                ptt = pp.tile([128, 128], FP16, tag="po", bufs=2)
                nc.tensor.transpose(ptt[:], vt_sbt[:, c4 * 128:(c4 + 1) * 128], ident[:])
                nc.vector.tensor_copy(v_sb[:, (4 * j + c4) * 128:(4 * j + c4 + 1) * 128], ptt[:])

            # ---- A2: Q^T per head + rope ----
            q_tiles = []
            for h in range(R):
                q_ps = pp.tile([128, TQ], F32, tag="pa", bufs=2)
                for d in range(ND):
                    nc.tensor.matmul(
                        q_ps[:], wq_sb[:, d * GC + h * 128:d * GC + (h + 1) * 128],
                        xts[d], start=(d == 0), stop=(d == ND - 1))
                qh = sb.tile([128, TQ], FP16, tag="qsb", bufs=5)
                rope(qh[:], q_ps, cosj, sinj)
                q_tiles.append(qh)

            # ---- B: causal attention per head (transposed S^T orientation),
            # interleaved with the previous tile's output projection so the
            # tensor engine has scalar-independent work during exp waits ----
            o_tiles = []
            nch = 4 * (j + 1)
            npair = nch // 2
            for h in range(R):
                o_ps = pp.tile([128, TQ], F32, tag="po", bufs=2)
                acc = sb.tile([128, TQ], FP16, tag="acc", bufs=2)
                for i in range(npair):
                    pair = ((0, 2 * i), (1, 2 * i + 1))
                    sp = pp.tile([128, 2 * TQ], F32, tag="sp", bufs=2)
                    p_sb = sb.tile([128, 2 * TQ], FP16, tag="psb", bufs=4)
                    for k, c in pair:
                        m = c - 4 * j
                        lo = m * 128 if m > 0 else 0
                        nc.tensor.matmul(sp[:, k * TQ + lo:(k + 1) * TQ],
                                         kT_sb[:, c * 128:(c + 1) * 128],
                                         q_tiles[h][:, lo:TQ], start=True, stop=True)
                    if pair[1][1] < 4 * j:  # both off-diagonal: one wide exp
                        nc.scalar.activation(p_sb[:], sp[:], AF.Exp, scale=SCALE)
                    else:
                        for k, c in pair:
                            m = c - 4 * j
                            lo = m * 128 if m > 0 else 0
                            nc.scalar.activation(p_sb[:, k * TQ + lo:(k + 1) * TQ],
                                                 sp[:, k * TQ + lo:(k + 1) * TQ],
                                                 AF.Exp, scale=SCALE)
                    for k, c in pair:
                        m = c - 4 * j
                        if m >= 0:  # diagonal block: triangle mask
                            blk = slice(k * TQ + m * 128, k * TQ + (m + 1) * 128)
                            nc.vector.tensor_mul(p_sb[:, blk], p_sb[:, blk], mask_sb[:])
                    # sigma: accumulate exp rows on DVE (partition-reduced later)
                    for k, c in pair:
                        m = c - 4 * j
                        lo = m * 128 if m > 0 else 0
                        if i == 0 and k == 0:
                            nc.vector.tensor_copy(acc[:], p_sb[:, 0:TQ])
                        else:
                            nc.vector.tensor_add(acc[:, lo:TQ], acc[:, lo:TQ],
                                                 p_sb[:, k * TQ + lo:(k + 1) * TQ])
                    for k, c in pair:
                        m = c - 4 * j
                        lo = m * 128 if m > 0 else 0
                        nc.tensor.matmul(o_ps[:, lo:TQ], v_sb[:, c * 128:(c + 1) * 128],
                                         p_sb[:, k * TQ + lo:(k + 1) * TQ],
                                         start=(i == 0 and k == 0),
                                         stop=(i == npair - 1 and k == 1))
                # partition-reduce sigma + broadcast via all-ones stationary
                sg_ps = pp.tile([128, TQ], F32, tag="po", bufs=2)
                nc.tensor.matmul(sg_ps[:], ones_c[:], acc[:], start=True, stop=True)
                rcb = sb.tile([128, TQ], F32, tag="rcb", bufs=2)
                nc.vector.reciprocal_approx_fast(rcb[:], sg_ps[:])
                oh = sb.tile([128, TQ], FP16, tag="osb", bufs=8)
                nc.vector.tensor_mul(oh[:], o_ps[:], rcb[:])
                o_tiles.append(oh)
                # previous q-tile's output projection: pure-tensor filler
                if prev_o is not None:
                    c_block(prev_o, prev_q0, h)
            prev_o, prev_q0 = o_tiles, q0

        # ---- C for the last q-tile ----
        for qs in range(4):
            c_block(prev_o, prev_q0, qs, split_dma=(qs == 3))

    nc.compile()
    return nc


def _get_nc():
    global _CACHED_NC
    if _CACHED_NC is None:
        _CACHED_NC = _build_nc()
    return _CACHED_NC


def _rope_tables_T():
    inv_freq = (1.0 / (THETA ** (np.arange(0, HD, 2, dtype=np.float32) / HD))).astype(np.float32)
    pos = np.arange(T, dtype=np.float32)
    freqs = np.outer(pos, inv_freq).astype(np.float32)      # [T, HD/2]
    emb = np.concatenate([freqs, freqs], axis=-1)           # [T, HD]
    return (np.cos(emb).T.astype(np.float16).copy(),
            np.sin(emb).T.astype(np.float16).copy())        # [HD, T]


def _tri_mask():
    # keep col >= row within a 128x128 diagonal block
    i = np.arange(128)[:, None]
    jj = np.arange(128)[None, :]
    return (jj >= i).astype(np.float16)


def kernel(x, Wq, Wk, Wv, Wo, _trace=False):
    x = np.asarray(x, dtype=np.float32)
    Wq = np.asarray(Wq, dtype=np.float16)
    Wk = np.asarray(Wk, dtype=np.float16)
    Wv = np.asarray(Wv, dtype=np.float16)
    Wo = np.asarray(Wo, dtype=np.float16)

    cosT, sinT = _rope_tables_T()
    trimask = _tri_mask()
    in_maps = []
    for core in range(8):
        b, g = core // KV, core % KV
        def chunkT(w):  # [ND*128, C] -> [128, ND*C] with chunk d at cols [d*C,(d+1)*C)
            nd = w.shape[0] // 128
            return np.ascontiguousarray(
                w.reshape(nd, 128, -1).transpose(1, 0, 2).reshape(128, -1))
        in_maps.append({
            "xT": np.ascontiguousarray(x[b].T.astype(np.float16)).reshape(ND, 128, T),
            "wq": chunkT(Wq[:, g * GC:(g + 1) * GC]),
            "wk": chunkT(Wk[:, g * HD:(g + 1) * HD]),
            "wv": chunkT(Wv[:, g * HD:(g + 1) * HD]),
            "wo": chunkT(Wo[g * GC:(g + 1) * GC, :]),
            "cosT": cosT, "sinT": sinT, "trimask": trimask,
        })

    nc = _get_nc()
    res = run_bass_kernel_spmd(nc, in_maps, core_ids=list(range(8)), trace=_trace)

    outp = np.zeros((B, T, D), dtype=np.float32)
    for core in range(8):
        b = core // KV
        outp[b] += res.results[core]["out"].astype(np.float32)
    if _trace:
        kernel._last_exec_time_ns = res.exec_time_ns
        kernel._last_trace = res.instructions_and_trace
    return outp


# revision 23
# speedup vs baseline: 1.0173x; 1.0173x over previous
"""GQA attention kernel for Trainium2, 8-core tensor-parallel.

Sharding: 8 cores = 2 batches x 4 KV-groups. Each core handles one
(batch, kv_group): projections for its 4 Q-heads + 1 KV-head, RoPE,
causal attention, and its row-shard of Wo -> partial [T, D] output.
Host sums the 4 partials per batch (the Wo all-reduce) at unshard.

V2: fp16 matmul operands throughout (same PE rate as f32r, half the
DMA/SBUF, 4x DVE modes). Attention in transposed orientation (S^T
tiles [s,q] from single K=128 matmuls). Softmax row-sums accumulate on
the vector engine into an SBUF f32 tile, reduced by ONE all-ones
stationary matmul per (j,head) which also broadcasts sums across
partitions. Diagonal blocks are trimmed to the causal region at
128-col granularity; the per-element causal mask is a single [128,128]
triangle multiply per diagonal block. exp() runs once per PAIR of
s-chunks over a 2-bank PSUM tile. PSUM->SBUF output copies run on the
Pool engine; output partials are written fp16 (host sums in f32).
"""
from contextlib import ExitStack

import numpy as np

import concourse.bass as bass
import concourse.mybir as mybir
import concourse.tile as tile
from concourse import bacc
from concourse.bass_utils import run_bass_kernel_spmd

B, T, D = 2, 2048, 2048
H, KV, HD = 16, 4, 128
R = H // KV                  # 4 query heads per kv head (per core)
GC = R * HD                  # 512 query-proj cols per core
THETA = 10000.0
TQ = 512                     # q-tile size
NJ = T // TQ                 # 4 q-tiles
ND = D // 128                # 16 contraction chunks
SCALE = float(HD) ** -0.5

F32 = mybir.dt.float32
F32R = mybir.dt.float32r
FP16 = mybir.dt.float16
AF = mybir.ActivationFunctionType

_CACHED_NC = None


def _build_nc():
    nc = bacc.Bacc("TRN2", target_bir_lowering=False, debug=False, num_devices=8)

    # xC: host-pretransposed activations, cols j*ND*TQ + d*TQ + c
    xC = nc.dram_tensor("xC", [128, NJ * ND * TQ], FP16, kind="ExternalInput").ap()
    wq = nc.dram_tensor("wq", [128, ND * GC], FP16, kind="ExternalInput").ap()
    wk = nc.dram_tensor("wk", [128, ND * HD], FP16, kind="ExternalInput").ap()
    wv = nc.dram_tensor("wv", [128, ND * HD], FP16, kind="ExternalInput").ap()
    wo = nc.dram_tensor("wo", [128, R * D], FP16, kind="ExternalInput").ap()
    cosT = nc.dram_tensor("cosT", [HD, T], FP16, kind="ExternalInput").ap()
    sinT = nc.dram_tensor("sinT", [HD, T], FP16, kind="ExternalInput").ap()
    trimask = nc.dram_tensor("trimask", [128, 128], FP16, kind="ExternalInput").ap()
    out = nc.dram_tensor("out", [T, D], FP16, kind="ExternalOutput").ap()

    with tile.TileContext(nc) as tc, ExitStack() as ctx:
        res = ctx.enter_context(tc.tile_pool(name="res", bufs=1))
        sb = ctx.enter_context(tc.tile_pool(name="sb", bufs=2))
        pp = ctx.enter_context(tc.tile_pool(name="pp", bufs=2, space="PSUM"))

        # ---- resident weights / tables ----
        # j=0 activations interleave with the weight DMAs in consumption
        # order so the first matmuls start as early as possible.
        # j=0 activations in 4 quarter DMAs (progressive arrival for A1/A2)
        wk_sb = res.tile([128, ND * HD], FP16)
        nc.sync.dma_start(wk_sb[:], wk[:])
        xt0_q = [sb.tile([128, 4 * TQ], FP16, tag="xtq", bufs=4, name=f"xt0_q{qtr}")
                 for qtr in range(4)]
        for qtr in range(4):
            nc.sync.dma_start(xt0_q[qtr][:],
                              xC[:, qtr * 4 * TQ:(qtr + 1) * 4 * TQ])
        cosj0 = sb.tile([128, TQ], FP16, tag="cos", bufs=2, name="cosj0")
        nc.sync.dma_start(cosj0[:], cosT[:, 0:TQ])
        sinj0 = sb.tile([128, TQ], FP16, tag="sin", bufs=2, name="sinj0")
        nc.sync.dma_start(sinj0[:], sinT[:, 0:TQ])
        wv_sb = res.tile([128, ND * HD], FP16)
        nc.sync.dma_start(wv_sb[:], wv[:])
        wq_sb = res.tile([128, ND * GC], FP16)    # chunk d at cols [d*GC, (d+1)*GC)
        for part in range(4):
            nc.sync.dma_start(wq_sb[:, part * 4 * GC:(part + 1) * 4 * GC],
                              wq[:, part * 4 * GC:(part + 1) * 4 * GC])
        mask_sb = res.tile([128, 128], FP16)
        nc.sync.dma_start(mask_sb[:], trimask[:])
        wo_sb = res.tile([128, R * D], FP16)      # head h rows at cols [h*D, (h+1)*D)
        nc.sync.dma_start(wo_sb[:], wo[:])
        kT_sb = res.tile([128, T], FP16)          # K^T resident, filled per j
        v_sb = res.tile([128, T], FP16)           # V natural, chunk c at cols c*128
        ident = res.tile([128, 128], FP16)
        from concourse.masks import make_identity
        make_identity(nc, ident[:])
        ones_c = res.tile([128, 128], FP16)       # sigma-reduce+broadcast stationary
        nc.vector.memset(ones_c[:], 1.0)

        def c_block(o_t, q0p, qs, split_dma=False):
            # output projection for rows [q0p+qs*128, q0p+(qs+1)*128)
            ob = sb.tile([128, D], FP16, tag="ob", bufs=3)
            for n in range(NJ):
                pc = pp.tile([128, 512], F32, tag="pa", bufs=2)
                for h2 in range(R):
                    nc.tensor.matmul(
                        pc[:], o_t[h2][:, qs * 128:(qs + 1) * 128],
                        wo_sb[:, h2 * D + n * 512:h2 * D + (n + 1) * 512],
                        start=(h2 == 0), stop=(h2 == R - 1))
                if n % 2 == 0:
                    nc.scalar.copy(ob[:, n * 512:(n + 1) * 512], pc[:])
                else:
                    nc.vector.tensor_copy(ob[:, n * 512:(n + 1) * 512], pc[:])
                if split_dma:
                    nc.gpsimd.dma_start(
                        out[q0p + qs * 128:q0p + (qs + 1) * 128,
                            n * 512:(n + 1) * 512],
                        ob[:, n * 512:(n + 1) * 512])
            if not split_dma:
                nc.gpsimd.dma_start(
                    out[q0p + qs * 128:q0p + (qs + 1) * 128, :], ob[:])

        def rope(dst, ps, cosj, sinj):
            # dst = ps * cos + rotate_half(ps) * sin   (partition dim = head dim)
            # one scalar op moves PSUM->SBUF fp16; the rest is 4x-mode DVE.
            ps_sb = sb.tile([128, TQ], FP16, tag="ps_sb", bufs=2)
            nc.scalar.copy(ps_sb[:], ps[:])
            rot = sb.tile([128, TQ], FP16, tag="rot", bufs=2)
            nc.vector.tensor_scalar_mul(rot[0:64, :], ps_sb[64:128, :], -1.0)
            nc.vector.tensor_copy(rot[64:128, :], ps_sb[0:64, :])
            tmp = sb.tile([128, TQ], FP16, tag="ropetmp", bufs=2)
            nc.vector.tensor_mul(tmp[:], rot[:], sinj[:])
            nc.vector.tensor_mul(dst, ps_sb[:], cosj[:])
            nc.vector.tensor_add(dst, dst, tmp[:])

        def stage(j):
            q0 = j * TQ
            if j == 0:
                return ([xt0_q[d // 4][:, (d % 4) * TQ:(d % 4 + 1) * TQ]
                         for d in range(ND)], cosj0, sinj0)
            xt_all = sb.tile([128, ND * TQ], FP16, tag="xt", bufs=2)
            nc.sync.dma_start(xt_all[:], xC[:, j * ND * TQ:(j + 1) * ND * TQ])
            xts = [xt_all[:, d * TQ:(d + 1) * TQ] for d in range(ND)]
            cosj = sb.tile([128, TQ], FP16, tag="cos", bufs=2)
            nc.sync.dma_start(cosj[:], cosT[:, q0:q0 + TQ])
            sinj = sb.tile([128, TQ], FP16, tag="sin", bufs=2)
            nc.sync.dma_start(sinj[:], sinT[:, q0:q0 + TQ])
            return xts, cosj, sinj

        def kv_proj(k_ps, vt_ps, xts, d0, d1):
            for d in range(d0, d1):
                nc.tensor.matmul(k_ps[:], wk_sb[:, d * HD:(d + 1) * HD], xts[d],
                                 start=(d == 0), stop=(d == ND - 1))
            for d in range(d0, d1):
                nc.tensor.matmul(vt_ps[:], wv_sb[:, d * HD:(d + 1) * HD], xts[d],
                                 start=(d == 0), stop=(d == ND - 1))

        def a1_finish(j, k_ps, vt_ps, cosj, sinj):
            q0 = j * TQ
            rope(kT_sb[:, q0:q0 + TQ], k_ps, cosj, sinj)
            vt_sbt = sb.tile([128, TQ], FP16, tag="vtsb", bufs=2)
            nc.scalar.copy(vt_sbt[:], vt_ps[:])
            for c4 in range(4):
                ptt = pp.tile([128, 128], FP16, tag="po", bufs=2)
                nc.tensor.transpose(ptt[:], vt_sbt[:, c4 * 128:(c4 + 1) * 128], ident[:])
                nc.vector.tensor_copy(
                    v_sb[:, (4 * j + c4) * 128:(4 * j + c4 + 1) * 128], ptt[:])

        prev_o, prev_q0 = None, 0
        st1 = kv1 = None
        for j in range(NJ):
            q0 = j * TQ
            if j == 1:
                xts, cosj, sinj = st1      # staged + A1 hoisted into B(0)
            else:
                xts, cosj, sinj = stage(j)
                # ---- A1: K^T and V^T for s-tile j ----
                k_ps = pp.tile([128, TQ], F32, tag="pa", bufs=2)
                vt_ps = pp.tile([128, TQ], F32, tag="pa", bufs=2)
                kv_proj(k_ps, vt_ps, xts, 0, ND)
                a1_finish(j, k_ps, vt_ps, cosj, sinj)

            # ---- A2: Q^T per head + rope ----
            q_tiles = []
            for h in range(R):
                q_ps = pp.tile([128, TQ], F32, tag="pa", bufs=2)
                for d in range(ND):
                    nc.tensor.matmul(
                        q_ps[:], wq_sb[:, d * GC + h * 128:d * GC + (h + 1) * 128],
                        xts[d], start=(d == 0), stop=(d == ND - 1))
                qh = sb.tile([128, TQ], FP16, tag="qsb", bufs=5)
                rope(qh[:], q_ps, cosj, sinj)
                q_tiles.append(qh)

            if j == 0:
                # stage j=1 now; its K/V projection becomes B(0)'s filler
                st1 = stage(1)
                kv1 = (pp.tile([128, TQ], F32, tag="pa", bufs=2, name="k_ps1"),
                       pp.tile([128, TQ], F32, tag="pa", bufs=2, name="vt_ps1"))

            # ---- B: causal attention per head (transposed S^T orientation),
            # interleaved with the previous tile's output projection so the
            # tensor engine has scalar-independent work during exp waits ----
            o_tiles = []
            nch = 4 * (j + 1)
            npair = nch // 2
            for h in range(R):
                o_ps = pp.tile([128, TQ], F32, tag="po", bufs=2)
                acc = sb.tile([128, TQ], FP16, tag="acc", bufs=2)
                for i in range(npair):
                    pair = ((0, 2 * i), (1, 2 * i + 1))
                    sp = pp.tile([128, 2 * TQ], F32, tag="sp", bufs=2)
                    p_sb = sb.tile([128, 2 * TQ], FP16, tag="psb", bufs=4)
                    for k, c in pair:
                        m = c - 4 * j
                        lo = m * 128 if m > 0 else 0
                        nc.tensor.matmul(sp[:, k * TQ + lo:(k + 1) * TQ],
                                         kT_sb[:, c * 128:(c + 1) * 128],
                                         q_tiles[h][:, lo:TQ], start=True, stop=True)
                    if pair[1][1] < 4 * j:  # both off-diagonal: one wide exp
                        nc.scalar.activation(p_sb[:], sp[:], AF.Exp, scale=SCALE)
                    else:
                        for k, c in pair:
                            m = c - 4 * j
                            lo = m * 128 if m > 0 else 0
                            nc.scalar.activation(p_sb[:, k * TQ + lo:(k + 1) * TQ],
                                                 sp[:, k * TQ + lo:(k + 1) * TQ],
                                                 AF.Exp, scale=SCALE)
                    for k, c in pair:
                        m = c - 4 * j
                        if m >= 0:  # diagonal block: triangle mask
                            blk = slice(k * TQ + m * 128, k * TQ + (m + 1) * 128)
                            nc.vector.tensor_mul(p_sb[:, blk], p_sb[:, blk], mask_sb[:])
                    # sigma: accumulate exp rows on DVE (partition-reduced later)
                    for k, c in pair:
                        m = c - 4 * j
                        lo = m * 128 if m > 0 else 0
                        if i == 0 and k == 0:
                            nc.vector.tensor_copy(acc[:], p_sb[:, 0:TQ])
                        else:
                            nc.vector.tensor_add(acc[:, lo:TQ], acc[:, lo:TQ],
                                                 p_sb[:, k * TQ + lo:(k + 1) * TQ])
                    for k, c in pair:
                        m = c - 4 * j
                        lo = m * 128 if m > 0 else 0
                        nc.tensor.matmul(o_ps[:, lo:TQ], v_sb[:, c * 128:(c + 1) * 128],
                                         p_sb[:, k * TQ + lo:(k + 1) * TQ],
                                         start=(i == 0 and k == 0),
                                         stop=(i == npair - 1 and k == 1))
                # partition-reduce sigma + broadcast via all-ones stationary
                sg_ps = pp.tile([128, TQ], F32, tag="po", bufs=2)
                nc.tensor.matmul(sg_ps[:], ones_c[:], acc[:], start=True, stop=True)
                rcb = sb.tile([128, TQ], F32, tag="rcb", bufs=2)
                nc.vector.reciprocal_approx_fast(rcb[:], sg_ps[:])
                oh = sb.tile([128, TQ], FP16, tag="osb", bufs=8)
                nc.vector.tensor_mul(oh[:], o_ps[:], rcb[:])
                o_tiles.append(oh)
                # pure-tensor filler: C(j-1) block, or j=1's K/V projection
                if prev_o is not None:
                    c_block(prev_o, prev_q0, h)
                else:
                    kv_proj(kv1[0], kv1[1], st1[0], h * 4, (h + 1) * 4)
            if j == 0:
                a1_finish(1, kv1[0], kv1[1], st1[1], st1[2])
            prev_o, prev_q0 = o_tiles, q0

        # ---- C for the last q-tile ----
        for qs in range(4):
            c_block(prev_o, prev_q0, qs, split_dma=(qs == 3))

    nc.compile()
    return nc


def _get_nc():
    global _CACHED_NC
    if _CACHED_NC is None:
        _CACHED_NC = _build_nc()
    return _CACHED_NC


def _rope_tables_T():
    inv_freq = (1.0 / (THETA ** (np.arange(0, HD, 2, dtype=np.float32) / HD))).astype(np.float32)
    pos = np.arange(T, dtype=np.float32)
    freqs = np.outer(pos, inv_freq).astype(np.float32)      # [T, HD/2]
    emb = np.concatenate([freqs, freqs], axis=-1)           # [T, HD]
    return (np.cos(emb).T.astype(np.float16).copy(),
            np.sin(emb).T.astype(np.float16).copy())        # [HD, T]


def _tri_mask():
    # keep col >= row within a 128x128 diagonal block
    i = np.arange(128)[:, None]
    jj = np.arange(128)[None, :]
    return (jj >= i).astype(np.float16)


def kernel(x, Wq, Wk, Wv, Wo, _trace=False):
    x = np.asarray(x, dtype=np.float32)
    Wq = np.asarray(Wq, dtype=np.float16)
    Wk = np.asarray(Wk, dtype=np.float16)
    Wv = np.asarray(Wv, dtype=np.float16)
    Wo = np.asarray(Wo, dtype=np.float16)

    cosT, sinT = _rope_tables_T()
    trimask = _tri_mask()
    in_maps = []
    for core in range(8):
        b, g = core // KV, core % KV
        def chunkT(w):  # [ND*128, C] -> [128, ND*C] with chunk d at cols [d*C,(d+1)*C)
            nd = w.shape[0] // 128
            return np.ascontiguousarray(
                w.reshape(nd, 128, -1).transpose(1, 0, 2).reshape(128, -1))
        # xC[p, j*ND*TQ + d*TQ + c] = x[b][j*TQ + c, d*128 + p]
        xc = (x[b].T.astype(np.float16)
              .reshape(ND, 128, NJ, TQ).transpose(1, 2, 0, 3)
              .reshape(128, NJ * ND * TQ))
        in_maps.append({
            "xC": np.ascontiguousarray(xc),
            "wq": chunkT(Wq[:, g * GC:(g + 1) * GC]),
            "wk": chunkT(Wk[:, g * HD:(g + 1) * HD]),
            "wv": chunkT(Wv[:, g * HD:(g + 1) * HD]),
            "wo": chunkT(Wo[g * GC:(g + 1) * GC, :]),
            "cosT": cosT, "sinT": sinT, "trimask": trimask,
        })

    nc = _get_nc()
    res = run_bass_kernel_spmd(nc, in_maps, core_ids=list(range(8)), trace=_trace)

    outp = np.zeros((B, T, D), dtype=np.float32)
    for core in range(8):
        b = core // KV
        outp[b] += res.results[core]["out"].astype(np.float32)
    if _trace:
        kernel._last_exec_time_ns = res.exec_time_ns
        kernel._last_trace = res.instructions_and_trace
    return outp
